# Initial kernel scaffold
#
"""Distributed single-head attention on 8 TRN2 NeuronCores.

Math (matches the reference):
    q = z @ Wq; k = z @ Wk; v = z @ Wv
    out = softmax(q k^T) * DK**-0.5 @ v

Sharding: z rows split 8 ways. Each core projects its own shard, the
K^T (fp16) and V (bf16) shards are all-gathered (one packed AllGather),
then each core does flash-style row-block attention:
    S^T_j = K^T[:, j-tile] ^T-matmul Q^T           (fp16 operands, f32 PSUM)
    P_j   = exp(S^T_j - 40)                        (bf16, shift-invariant)
    rowsumT = ones^T @ P                           (PE)
    out   = (P^T-matmuls V) * (scale / rowsum)
Layouts keep seq on partitions for P so both matmuls are native
(no transposes anywhere; z^T is prepared on the host).

Precision: fp16 z/W/Q/K + f32 PSUM keeps logits to ~1e-2 abs err;
exp/V/AV in bf16. End-to-end rel err ~3e-3 (vs f32 reference).
"""

import numpy as np

SEQ, D, DK, DV = 4096, 1024, 1024, 1024
NCORES = 8
ROWS = SEQ // NCORES            # 512 rows per core
DT = D // 128                   # 8 contraction tiles (input dim)
MT = DK // 128                  # 8 dk tiles
ST = ROWS // 128                # 4 local seq tiles
JT = SEQ // 128                 # 32 global seq tiles
SHIFT = 40.0                    # constant logit shift (softmax-invariant)
SCALE = DK ** -0.5

KT_ELEMS = DK * ROWS            # fp16 K^T shard elems in packed bounce
PACK_ELEMS = KT_ELEMS + ROWS * DV


def _build():
    import concourse.mybir as mybir
    import concourse.tile as tile
    from concourse import bacc

    F32 = mybir.dt.float32
    F16 = mybir.dt.float16
    BF16 = mybir.dt.bfloat16
    Exp = mybir.ActivationFunctionType.Exp

    nc = bacc.Bacc(None, target_bir_lowering=False, debug=False)
    d_zT = nc.declare_dram_parameter("zT", [D, ROWS], F16, isOutput=False)
    d_wq = nc.declare_dram_parameter("Wq", [D, DK], F16, isOutput=False)
    d_wk = nc.declare_dram_parameter("Wk", [D, DK], F16, isOutput=False)
    d_wv = nc.declare_dram_parameter("Wv", [D, DV], F16, isOutput=False)
    d_out = nc.declare_dram_parameter("out", [ROWS, DV], F32, isOutput=True)

    with tile.TileContext(nc) as tc:
        with (
            tc.tile_pool(name="dram", bufs=1, space="DRAM") as dram,
            tc.tile_pool(name="qt", bufs=1) as qt_pool,
            tc.tile_pool(name="misc", bufs=1) as misc,
            tc.tile_pool(name="stage", bufs=4) as stage,
            tc.tile_pool(name="ps_proj", bufs=2, space="PSUM") as ps_proj,
            tc.tile_pool(name="ps_s", bufs=2, space="PSUM") as ps_s,
            tc.tile_pool(name="ps_rs", bufs=1, space="PSUM") as ps_rs,
            tc.tile_pool(name="ps_o", bufs=2, space="PSUM") as ps_o,
            tc.tile_pool(name="outp", bufs=2) as outp,
        ):
            # ---- packed bounce + collective buffers (bf16-typed bytes) ----
            pack_in = dram.tile([PACK_ELEMS], BF16)
            pack_out = dram.tile([NCORES * PACK_ELEMS], BF16, addr_space="Shared")

            ones_sb = misc.tile([128, 1], BF16)
            nc.vector.memset(ones_sb[:], 1.0)

            # ---------------- projection phase (scoped weights) ----------
            with tc.tile_pool(name="wz", bufs=1) as wz:
                zT_sb = wz.tile([128, DT, ROWS], F16)
                zv = d_zT.rearrange("(t p) n -> p t n", p=128)
                for h in range(2):
                    nc.sync.dma_start(zT_sb[:, h * 4:(h + 1) * 4, :],
                                      zv[:, h * 4:(h + 1) * 4, :])
                wk_sb = wz.tile([128, DT, DK], F16)
                wv_sb = wz.tile([128, DT, DV], F16)
                wq_sb = wz.tile([128, DT, DK], F16)
                for t in range(DT):
                    nc.sync.dma_start(
                        wk_sb[:, t, :],
                        d_wk.rearrange("(t p) m -> p t m", p=128)[:, t, :])
                for t in range(DT):
                    nc.sync.dma_start(
                        wv_sb[:, t, :],
                        d_wv.rearrange("(t p) m -> p t m", p=128)[:, t, :])
                for t in range(DT):
                    nc.sync.dma_start(
                        wq_sb[:, t, :],
                        d_wq.rearrange("(t p) m -> p t m", p=128)[:, t, :])

                # K^T shard: [DK, ROWS] fp16 -> pack_in[0:KT_ELEMS]
                ktv = pack_in[0:KT_ELEMS].rearrange("(m p n) -> p m n", p=128, n=ROWS)
                for m in range(MT):
                    pk = ps_proj.tile([128, 512], F32, tag="psproj")
                    for t in range(DT):
                        nc.tensor.matmul(pk[:], wk_sb[:, t, m * 128:(m + 1) * 128],
                                         zT_sb[:, t, :],
                                         start=(t == 0), stop=(t == DT - 1))
                    kt_stage = stage.tile([128, ROWS], F16, tag="ktstage")
                    nc.vector.tensor_copy(kt_stage[:], pk[:])
                    nc.sync.dma_start(ktv[:, m, :], kt_stage[:].bitcast(BF16))

                # V shard: [ROWS, DV] bf16 -> pack_in[KT_ELEMS:]
                vv = pack_in[KT_ELEMS:PACK_ELEMS].rearrange(
                    "(s p m) -> p s m", p=128, m=DV)
                for s in range(ST):
                    for h in range(2):
                        pv = ps_proj.tile([128, 512], F32, tag="psproj")
                        for t in range(DT):
                            nc.tensor.matmul(
                                pv[:], zT_sb[:, t, s * 128:(s + 1) * 128],
                                wv_sb[:, t, h * 512:(h + 1) * 512],
                                start=(t == 0), stop=(t == DT - 1))
                        v_stage = stage.tile([128, 512], BF16, tag="vstage")
                        nc.vector.tensor_copy(v_stage[:], pv[:])
                        nc.sync.dma_start(vv[:, s, h * 512:(h + 1) * 512], v_stage[:])

                # all-gather the packed K^T/V shards (TOPSP; PE keeps going)
                cc_sem = None
                nc.gpsimd.collective_compute(
                    "AllGather",
                    mybir.AluOpType.bypass,
                    replica_groups=[list(range(NCORES))],
                    ins=[pack_in[:].opt()],
                    outs=[pack_out[:].opt()],
                )

                # Q^T: [DK, ROWS] fp16, resident (overlaps the collective)
                qt_sb = qt_pool.tile([128, MT, ROWS], F16)
                for m in range(MT):
                    pq = ps_proj.tile([128, 512], F32, tag="psproj")
                    for t in range(DT):
                        nc.tensor.matmul(pq[:], wq_sb[:, t, m * 128:(m + 1) * 128],
                                         zT_sb[:, t, :],
                                         start=(t == 0), stop=(t == DT - 1))
                    nc.vector.tensor_copy(qt_sb[:, m, :], pq[:])

            # ---------------- gathered tiles ------------------------------
            with (
                tc.tile_pool(name="ktg", bufs=4) as ktg_pool,
                tc.tile_pool(name="vg", bufs=1) as vg_pool,
                tc.tile_pool(name="expp", bufs=1) as expp,
            ):
                # V gathered: resident [128, JT, DV] bf16 (64KB/partition)
                v_sb = vg_pool.tile([128, JT, DV], BF16)
                for b in range(NCORES):
                    src = pack_out[b * PACK_ELEMS + KT_ELEMS:
                                   (b + 1) * PACK_ELEMS].rearrange(
                        "(s p m) -> p s m", p=128, m=DV)
                    nc.sync.dma_start(v_sb[:, b * ST:(b + 1) * ST, :], src)

                expS = expp.tile([128, JT, ROWS], BF16)
                rs_ps = ps_rs.tile([1, 512], F32)

                kt_blocks = []
                for b in range(NCORES):
                    ktb = ktg_pool.tile([128, MT, ROWS], F16, tag="ktg")
                    src = pack_out[b * PACK_ELEMS:
                                   b * PACK_ELEMS + KT_ELEMS].rearrange(
                        "(m p n) -> p m n", p=128, n=ROWS).bitcast(F16)
                    for h in range(2):
                        nc.sync.dma_start(ktb[:, h * 4:(h + 1) * 4, :],
                                          src[:, h * 4:(h + 1) * 4, :])
                    kt_blocks.append(ktb)

                    # S-phase for the 4 j-tiles of this block
                    for jj in range(ST):
                        j = b * ST + jj
                        ps_S = ps_s.tile([128, 512], F32, tag="pss")
                        for t in range(MT):
                            nc.tensor.matmul(
                                ps_S[:],
                                ktb[:, t, jj * 128:(jj + 1) * 128],
                                qt_sb[:, t, :],
                                start=(t == 0), stop=(t == MT - 1))
                        nc.scalar.activation(expS[:, j, :], ps_S[:], Exp,
                                             bias=-SHIFT, scale=1.0)
                        nc.tensor.matmul(rs_ps[:], ones_sb[:], expS[:, j, :],
                                         start=(j == 0), stop=(j == JT - 1))

                # row-sum -> per-row reciprocal multipliers [128, ST]
                rs_sb = misc.tile([1, 512], F32)
                nc.vector.tensor_copy(rs_sb[:], rs_ps[:])
                rs_dram = dram.tile([1, 512], F32)
                nc.sync.dma_start(rs_dram[:], rs_sb[:])
                rs128 = misc.tile([128, ST], F32)
                nc.sync.dma_start(
                    rs128[:], rs_dram[0, :].rearrange("(r p) -> p r", p=128))
                mult_sb = misc.tile([128, ST], F32)
                nc.vector.reciprocal(mult_sb[:], rs128[:])
                nc.vector.tensor_scalar_mul(mult_sb[:], mult_sb[:], SCALE)

                # ---------------- AV phase ---------------------------------
                for h in range(2):
                    for r in range(ST):
                        po = ps_o.tile([128, 512], F32, tag="pso")
                        for j in range(JT):
                            nc.tensor.matmul(
                                po[:],
                                expS[:, j, r * 128:(r + 1) * 128],
                                v_sb[:, j, h * 512:(h + 1) * 512],
                                start=(j == 0), stop=(j == JT - 1))
                        o_sb = outp.tile([128, 512], F32, tag="osb")
                        nc.vector.tensor_scalar_mul(o_sb[:], po[:],
                                                    mult_sb[:, r:r + 1])
                        nc.sync.dma_start(
                            d_out[r * 128:(r + 1) * 128, h * 512:(h + 1) * 512],
                            o_sb[:])
    return nc


_BUILT = None


def kernel(z, Wq, Wk, Wv):
    global _BUILT
    from concourse.bass_utils import run_bass_kernel_spmd

    if _BUILT is None:
        _BUILT = _build()
    nc = _BUILT

    zT = np.ascontiguousarray(z.T).astype(np.float16)
    wq16 = Wq.astype(np.float16)
    wk16 = Wk.astype(np.float16)
    wv16 = Wv.astype(np.float16)
    in_maps = [
        {
            "zT": np.ascontiguousarray(zT[:, c * ROWS:(c + 1) * ROWS]),
            "Wq": wq16,
            "Wk": wk16,
            "Wv": wv16,
        }
        for c in range(NCORES)
    ]
    res = run_bass_kernel_spmd(nc, in_maps, list(range(NCORES)))
    out = np.concatenate([res.results[c]["out"] for c in range(NCORES)], axis=0)
    return out.astype(np.float32)


if __name__ == "__main__":
    rng = np.random.default_rng(0)
    z = rng.standard_normal((SEQ, D)).astype(np.float32)
    Wq = (0.02 * rng.standard_normal((D, DK))).astype(np.float32)
    Wk = (0.02 * rng.standard_normal((D, DK))).astype(np.float32)
    Wv = (0.02 * rng.standard_normal((D, DV))).astype(np.float32)
    out = kernel(z=z, Wq=Wq, Wk=Wk, Wv=Wv)
    print(out.shape, out.dtype)


# revision 5
# speedup vs baseline: 1.0045x; 1.0045x over previous
"""Distributed single-head attention on 8 TRN2 NeuronCores.

Math (matches the reference):
    q = z @ Wq; k = z @ Wk; v = z @ Wv
    out = softmax(q k^T) * DK**-0.5 @ v

Sharding: z rows split 8 ways. Each core projects its own shard, the
K^T (fp16) and V (bf16) shards are all-gathered (one packed AllGather),
then each core does flash-style row-block attention:
    S^T_j = K^T[:, j-tile] ^T-matmul Q^T           (fp16 operands, f32 PSUM)
    P_j   = exp(S^T_j - 40)                        (bf16, shift-invariant)
    rowsumT = ones^T @ P                           (PE)
    out   = (P^T-matmuls V) * (scale / rowsum)
Layouts keep seq on partitions for P so both matmuls are native
(no transposes anywhere; z^T is prepared on the host).

Precision: fp16 z/W/Q/K + f32 PSUM keeps logits to ~1e-2 abs err;
exp/V/AV in bf16. End-to-end rel err ~3e-3 (vs f32 reference).
"""

import numpy as np

SEQ, D, DK, DV = 4096, 1024, 1024, 1024
NCORES = 8
ROWS = SEQ // NCORES            # 512 rows per core
DT = D // 128                   # 8 contraction tiles (input dim)
MT = DK // 128                  # 8 dk tiles
ST = ROWS // 128                # 4 local seq tiles
JT = SEQ // 128                 # 32 global seq tiles
SHIFT = 40.0                    # constant logit shift (softmax-invariant)
SCALE = DK ** -0.5

KT_ELEMS = DK * ROWS            # fp16 K^T shard elems in packed bounce
PACK_ELEMS = KT_ELEMS + ROWS * DV


def _build():
    import concourse.mybir as mybir
    import concourse.tile as tile
    from concourse import bacc

    F32 = mybir.dt.float32
    F16 = mybir.dt.float16
    BF16 = mybir.dt.bfloat16
    Exp = mybir.ActivationFunctionType.Exp

    nc = bacc.Bacc(None, target_bir_lowering=False, debug=False)
    d_zT = nc.declare_dram_parameter("zT", [D, ROWS], F16, isOutput=False)
    d_wq = nc.declare_dram_parameter("Wq", [D, DK], F16, isOutput=False)
    d_wk = nc.declare_dram_parameter("Wk", [D, DK], F16, isOutput=False)
    d_wv = nc.declare_dram_parameter("Wv", [D, DV], F16, isOutput=False)
    d_out = nc.declare_dram_parameter("out", [ROWS, DV], F32, isOutput=True)

    with tile.TileContext(nc) as tc:
        with (
            tc.tile_pool(name="dram", bufs=1, space="DRAM") as dram,
            tc.tile_pool(name="qt", bufs=1) as qt_pool,
            tc.tile_pool(name="misc", bufs=1) as misc,
            tc.tile_pool(name="stage", bufs=4) as stage,
            tc.tile_pool(name="ps_proj", bufs=2, space="PSUM") as ps_proj,
            tc.tile_pool(name="ps_s", bufs=2, space="PSUM") as ps_s,
            tc.tile_pool(name="ps_rs", bufs=1, space="PSUM") as ps_rs,
            tc.tile_pool(name="ps_o", bufs=2, space="PSUM") as ps_o,
            tc.tile_pool(name="outp", bufs=2) as outp,
        ):
            # ---- packed bounce + collective buffers (bf16-typed bytes) ----
            pack_in = dram.tile([PACK_ELEMS], BF16)
            pack_out = dram.tile([NCORES * PACK_ELEMS], BF16, addr_space="Shared")

            ones_sb = misc.tile([128, 1], BF16)
            nc.vector.memset(ones_sb[:], 1.0)
            bias_sb = misc.tile([128, 1], F32)
            nc.vector.memset(bias_sb[:], -SHIFT)

            # ---------------- projection phase (scoped weights) ----------
            with tc.tile_pool(name="wz", bufs=1) as wz:
                zT_sb = wz.tile([128, DT, ROWS], F16)
                zv = d_zT.rearrange("(t p) n -> p t n", p=128)
                for h in range(2):
                    nc.sync.dma_start(zT_sb[:, h * 4:(h + 1) * 4, :],
                                      zv[:, h * 4:(h + 1) * 4, :])
                wk_sb = wz.tile([128, DT, DK], F16)
                wv_sb = wz.tile([128, DT, DV], F16)
                wq_sb = wz.tile([128, DT, DK], F16)
                for t in range(DT):
                    nc.sync.dma_start(
                        wk_sb[:, t, :],
                        d_wk.rearrange("(t p) m -> p t m", p=128)[:, t, :])
                for t in range(DT):
                    nc.sync.dma_start(
                        wv_sb[:, t, :],
                        d_wv.rearrange("(t p) m -> p t m", p=128)[:, t, :])
                for t in range(DT):
                    nc.sync.dma_start(
                        wq_sb[:, t, :],
                        d_wq.rearrange("(t p) m -> p t m", p=128)[:, t, :])

                # K^T shard: [DK, ROWS] fp16 -> pack_in[0:KT_ELEMS]
                ktv = pack_in[0:KT_ELEMS].rearrange("(m p n) -> p m n", p=128, n=ROWS)
                for m in range(MT):
                    pk = ps_proj.tile([128, 512], F32, tag="psproj")
                    for t in range(DT):
                        nc.tensor.matmul(pk[:], wk_sb[:, t, m * 128:(m + 1) * 128],
                                         zT_sb[:, t, :],
                                         start=(t == 0), stop=(t == DT - 1))
                    kt_stage = stage.tile([128, ROWS], F16, tag="ktstage")
                    nc.vector.tensor_copy(kt_stage[:], pk[:])
                    nc.sync.dma_start(ktv[:, m, :], kt_stage[:].bitcast(BF16))

                # V shard: [ROWS, DV] bf16 -> pack_in[KT_ELEMS:]
                vv = pack_in[KT_ELEMS:PACK_ELEMS].rearrange(
                    "(s p m) -> p s m", p=128, m=DV)
                for s in range(ST):
                    for h in range(2):
                        pv = ps_proj.tile([128, 512], F32, tag="psproj")
                        for t in range(DT):
                            nc.tensor.matmul(
                                pv[:], zT_sb[:, t, s * 128:(s + 1) * 128],
                                wv_sb[:, t, h * 512:(h + 1) * 512],
                                start=(t == 0), stop=(t == DT - 1))
                        v_stage = stage.tile([128, 512], BF16, tag="vstage")
                        nc.vector.tensor_copy(v_stage[:], pv[:])
                        nc.sync.dma_start(vv[:, s, h * 512:(h + 1) * 512], v_stage[:])

                # all-gather the packed K^T/V shards (TOPSP; PE keeps going)
                cc_sem = None
                nc.gpsimd.collective_compute(
                    "AllGather",
                    mybir.AluOpType.bypass,
                    replica_groups=[list(range(NCORES))],
                    ins=[pack_in[:].opt()],
                    outs=[pack_out[:].opt()],
                )

                # Q^T: [DK, ROWS] fp16, resident (overlaps the collective)
                qt_sb = qt_pool.tile([128, MT, ROWS], F16)
                for m in range(MT):
                    pq = ps_proj.tile([128, 512], F32, tag="psproj")
                    for t in range(DT):
                        nc.tensor.matmul(pq[:], wq_sb[:, t, m * 128:(m + 1) * 128],
                                         zT_sb[:, t, :],
                                         start=(t == 0), stop=(t == DT - 1))
                    nc.vector.tensor_copy(qt_sb[:, m, :], pq[:])

            # ---------------- gathered tiles ------------------------------
            with (
                tc.tile_pool(name="ktg", bufs=4) as ktg_pool,
                tc.tile_pool(name="vg", bufs=1) as vg_pool,
                tc.tile_pool(name="expp", bufs=1) as expp,
            ):
                # V gathered: resident [128, JT, DV] bf16 (64KB/partition)
                v_sb = vg_pool.tile([128, JT, DV], BF16)
                expS = expp.tile([128, JT, ROWS], BF16)
                rs_ps = ps_rs.tile([1, 512], F32)

                for b in range(NCORES):
                    ktb = ktg_pool.tile([128, MT, ROWS], F16, tag="ktg")
                    src = pack_out[b * PACK_ELEMS:
                                   b * PACK_ELEMS + KT_ELEMS].rearrange(
                        "(m p n) -> p m n", p=128, n=ROWS).bitcast(F16)
                    for h in range(2):
                        nc.sync.dma_start(ktb[:, h * 4:(h + 1) * 4, :],
                                          src[:, h * 4:(h + 1) * 4, :])
                    vsrc = pack_out[b * PACK_ELEMS + KT_ELEMS:
                                    (b + 1) * PACK_ELEMS].rearrange(
                        "(s p m) -> p s m", p=128, m=DV)
                    nc.sync.dma_start(v_sb[:, b * ST:(b + 1) * ST, :], vsrc)

                    # S-phase for the 4 j-tiles of this block
                    for jj in range(ST):
                        j = b * ST + jj
                        ps_S = ps_s.tile([128, 512], F32, tag="pss")
                        for t in range(MT):
                            nc.tensor.matmul(
                                ps_S[:],
                                ktb[:, t, jj * 128:(jj + 1) * 128],
                                qt_sb[:, t, :],
                                start=(t == 0), stop=(t == MT - 1))
                        nc.scalar.activation(expS[:, j, :], ps_S[:], Exp,
                                             bias=bias_sb[:], scale=1.0)
                        nc.tensor.matmul(rs_ps[:], ones_sb[:], expS[:, j, :],
                                         start=(j == 0), stop=(j == JT - 1))

                # row-sum -> per-row reciprocal multipliers [128, ST]
                rs_sb = misc.tile([1, 512], F32)
                nc.vector.tensor_copy(rs_sb[:], rs_ps[:])
                rs_dram = dram.tile([1, 512], F32)
                nc.sync.dma_start(rs_dram[:], rs_sb[:])
                rs128 = misc.tile([128, ST], F32)
                nc.sync.dma_start(
                    rs128[:], rs_dram[0, :].rearrange("(r p) -> p r", p=128))
                mult_sb = misc.tile([128, ST], F32)
                nc.vector.reciprocal(mult_sb[:], rs128[:])
                nc.vector.tensor_scalar_mul(mult_sb[:], mult_sb[:], SCALE)

                # ---------------- AV phase ---------------------------------
                for h in range(2):
                    for r in range(ST):
                        po = ps_o.tile([128, 512], F32, tag="pso")
                        for j in range(JT):
                            nc.tensor.matmul(
                                po[:],
                                expS[:, j, r * 128:(r + 1) * 128],
                                v_sb[:, j, h * 512:(h + 1) * 512],
                                start=(j == 0), stop=(j == JT - 1))
                        o_sb = outp.tile([128, 512], F32, tag="osb")
                        nc.vector.tensor_scalar_mul(o_sb[:], po[:],
                                                    mult_sb[:, r:r + 1])
                        nc.sync.dma_start(
                            d_out[r * 128:(r + 1) * 128, h * 512:(h + 1) * 512],
                            o_sb[:])
    nc.compile()
    return nc


_BUILT = None


def kernel(z, Wq, Wk, Wv):
    global _BUILT
    from concourse.bass_utils import run_bass_kernel_spmd

    if _BUILT is None:
        _BUILT = _build()
    nc = _BUILT

    zT = np.ascontiguousarray(z.T).astype(np.float16)
    wq16 = Wq.astype(np.float16)
    wk16 = Wk.astype(np.float16)
    wv16 = Wv.astype(np.float16)
    in_maps = [
        {
            "zT": np.ascontiguousarray(zT[:, c * ROWS:(c + 1) * ROWS]),
            "Wq": wq16,
            "Wk": wk16,
            "Wv": wv16,
        }
        for c in range(NCORES)
    ]
    res = run_bass_kernel_spmd(nc, in_maps, list(range(NCORES)))
    out = np.concatenate([res.results[c]["out"] for c in range(NCORES)], axis=0)
    return out.astype(np.float32)


if __name__ == "__main__":
    rng = np.random.default_rng(0)
    z = rng.standard_normal((SEQ, D)).astype(np.float32)
    Wq = (0.02 * rng.standard_normal((D, DK))).astype(np.float32)
    Wk = (0.02 * rng.standard_normal((D, DK))).astype(np.float32)
    Wv = (0.02 * rng.standard_normal((D, DV))).astype(np.float32)
    out = kernel(z=z, Wq=Wq, Wk=Wk, Wv=Wv)
    print(out.shape, out.dtype)
